# Initial kernel scaffold
#
"""Trainium2 Bass kernel for nn_DynamicCombiner (retrieval-kNN combiner).

Computes, per query row n (of N=2048, sharded 256 rows x 8 cores):
    ctx    = mean_k searched_hidden[n]                  [D]
    feat   = [hidden[n], ctx]                           [2D]
    bw     = exp(feat . bw_w + bw_b)
    w      = softmax(-dist[n]/bw)                       [K]
    mhid   = relu(feat @ mw_w1.T + mw_b1)
    mix    = sigmoid(mhid . mw_w2 + mw_b2)
    p      = softmax(logits[n])                         [V]
    out    = log((1-mix)*p + mix*scatter(w at tok[n]) + 1e-10)

Device strategy per core (R=256 rows = 2 partition-tiles of 128):
  - phase B: stream searched_hidden through a shared 128KB/partition SBUF
    slot, DVE-reduce over K -> ctx; TensorE-transpose h and ctx into
    feat^T (the 1/K mean scaling is folded into the host-side weights).
  - phase C: f32 TensorE matmuls for the tiny MLP + the two dot products;
    ACT handles relu/exp; sigmoid is computed as em/(1+em) on DVE to stay
    in the exp/ln ACT table set.
  - phase D: logits row-tile resident in the shared slot; ACT Exp pass with
    accum_out gives Z for free; the vocab-scatter correction is added in
    exp-space (corr = mix*W*Z/(1-mix), so Ln(scale*(t+corr)+eps) lands on
    log((1-mix)p + mix*W + eps)) via gpsimd local_scatter of the f32 values
    as int16 halves into a per-chunk buffer (host pre-builds per-1000-chunk
    relative indices, with out-of-chunk and duplicate slots negative);
    ACT Ln(a*t + eps) with per-partition scale a=(1-mix)/Z then writes the
    final dense values; DMA out. Duplicate token indices get their combined
    weight via a [K,K] is_equal matrix on DVE.
"""

import numpy as np

B, S, D, V, K = 8, 256, 1024, 32000, 32
N = B * S
NCORES = 8
R = N // NCORES  # rows per core
P = 128
T = R // P       # row-tiles per core
F = 2 * D
CH = 8000        # vocab chunk for streaming passes
NCH = V // CH
SC = 1000        # local_scatter sub-chunk (f32 elems; 2000 int16 < 64Ki/32)
NSC = V // SC    # 32 sub-chunks
SPC = CH // SC   # 8 sub-chunks per streaming chunk
DC = D // P      # 8 d-chunks
FC = F // P      # 16 feature chunks
EPS = 1e-10

_NC = {}


def _build_nc(reps=1, skip=()):
    import concourse.bacc as bacc
    import concourse.bass as bass
    import concourse.mybir as mybir
    import concourse.tile as tile
    from concourse.masks import make_identity

    fp32 = mybir.dt.float32
    i32 = mybir.dt.int32
    Alu = mybir.AluOpType
    Act = mybir.ActivationFunctionType

    nc = bacc.Bacc("TRN2", target_bir_lowering=False, debug=False,
                   num_devices=NCORES)

    lg = nc.dram_tensor("lg", [R, V], fp32, kind="ExternalInput")
    hid = nc.dram_tensor("hid", [R, D], fp32, kind="ExternalInput")
    dist = nc.dram_tensor("dist", [R, K], fp32, kind="ExternalInput")
    tok = nc.dram_tensor("tok", [R, K], i32, kind="ExternalInput")
    chx = nc.dram_tensor("chx", [P, T, NSC, 2 * K], mybir.dt.int16,
                         kind="ExternalInput")
    sh = nc.dram_tensor("sh", [R, K, D], fp32, kind="ExternalInput")
    w1t = nc.dram_tensor("w1t", [F, D], fp32, kind="ExternalInput")
    b1t = nc.dram_tensor("b1t", [P, DC], fp32, kind="ExternalInput")
    bwt = nc.dram_tensor("bwt", [F, 1], fp32, kind="ExternalInput")
    w2t = nc.dram_tensor("w2t", [D, 1], fp32, kind="ExternalInput")
    cvec = nc.dram_tensor("cvec", [1, 2], fp32, kind="ExternalInput")
    out = nc.dram_tensor("out", [R, V], fp32, kind="ExternalOutput")

    with tile.TileContext(nc) as tc:
        with (
            tc.tile_pool(name="bigp", bufs=1) as bigp,
            tc.tile_pool(name="sbp", bufs=1) as sbp,
            tc.tile_pool(name="psp", bufs=2, space="PSUM") as psp,
        ):
            # --- static tiles ---
            big = bigp.tile([P, K * D], fp32)          # 128KB/partition slot
            sh3 = big.rearrange("p (k d) -> p k d", k=K)
            featT = sbp.tile([P, FC, R], fp32, tag="mid")
            mhT = sbp.tile([P, DC, R], fp32)
            ident = sbp.tile([P, P], fp32)
            make_identity(nc, ident[:, :])
            epsb = sbp.tile([P, 1], fp32)
            nc.gpsimd.memset(epsb[:], EPS)

            idxi = sbp.tile([P, T, K], i32)
            idxf = sbp.tile([P, T, K], fp32)
            distf = sbp.tile([P, T, K], fp32)
            wks = sbp.tile([P, T, K], fp32)
            wprime = sbp.tile([P, T, K], fp32)
            cw = sbp.tile([P, T, K], fp32)
            chidx = sbp.tile([P, T, NSC, 2 * K], mybir.dt.int16)
            rav = sbp.tile([P, T], fp32)

            b1sb = sbp.tile([P, DC], fp32)
            bwsb = sbp.tile([P, FC], fp32)
            w2sb = sbp.tile([P, DC], fp32)
            cld = sbp.tile([P, 2], fp32)
            cbc = sbp.tile([P, 2], fp32)

            ch = CH if reps == 1 else 4000
            nchv = V // ch
            spc = ch // SC
            scc = 4000 if reps == 1 else 2000
            corr_static = (None if reps == 1 else
                           sbp.tile([P, 2, scc], fp32, name="corrs"))
            zp = sbp.tile([P, T * nchv], fp32)
            Zv = sbp.tile([P, T], fp32)
            Zi = sbp.tile([P, T], fp32)
            bwv = sbp.tile([P, T], fp32)
            rbw = sbp.tile([P, T], fp32)
            sev = sbp.tile([P, T], fp32)
            rse = sbp.tile([P, T], fp32)
            emv = sbp.tile([P, T], fp32)
            sden = sbp.tile([P, T], fp32)
            omv = sbp.tile([P, T], fp32)   # 1 - mix
            mv = sbp.tile([P, T], fp32)    # mix
            av = sbp.tile([P, T], fp32)    # (1-mix)/Z

            # --- small loads (ACT hwdge ring keeps them off the big stream) ---
            nc.scalar.dma_start(out=idxi[:], in_=tok[:, :].rearrange(
                "(t p) k -> p t k", p=P))
            nc.scalar.dma_start(out=distf[:], in_=dist[:, :].rearrange(
                "(t p) k -> p t k", p=P))
            nc.scalar.dma_start(out=chidx[:], in_=chx[:, :, :, :])
            nc.scalar.dma_start(out=b1sb[:], in_=b1t[:, :])
            nc.scalar.dma_start(out=bwsb[:], in_=bwt[:, 0].rearrange(
                "(c p) -> p c", p=P))
            nc.scalar.dma_start(out=w2sb[:], in_=w2t[:, 0].rearrange(
                "(c p) -> p c", p=P))
            nc.scalar.dma_start(out=cld[:1, :], in_=cvec[:, :])
            nc.gpsimd.partition_broadcast(cbc[:], cld[:1, :])

            nc.vector.tensor_copy(idxf[:], idxi[:])

            skipset = set(skip)
            anchors = None
            for rep in range(reps):
                env = dict(locals())
                env["skipset"] = skipset
                if anchors is None:
                    anchors = _emit_body(nc, tc, sbp, psp, mybir, Alu, Act, env)
                else:
                    # Hard barrier between reps: every instruction of this rep
                    # syncs on the previous rep's final store. Prevents the
                    # scheduler from hoisting slot allocators across the rep
                    # boundary (which deadlocks bufs=1 tags).
                    prev_inst = anchors["last"].ins

                    def _barrier_cb(ins_, _prev=prev_inst):
                        tile.add_dep_helper(ins_, _prev, sync=True,
                                            reason="rep barrier")

                    nc._state.push_inst_callback(_barrier_cb)
                    try:
                        anchors = _emit_body(nc, tc, sbp, psp, mybir, Alu,
                                             Act, env)
                    finally:
                        nc._state.remove_inst_callback(_barrier_cb)

    nc.compile()
    return nc


def _emit_body(nc, tc, sbp, psp, mybir, Alu, Act, env):
    import concourse.tile as tile_mod

    prev = env.get("anchors") or {}

    def bdep(inst, key="last"):
        # Cross-rep ordering: tie this rep's first allocator of a bufs-limited
        # tag to the previous rep's instruction whose read releases that tag's
        # slot. Without this the scheduler can order the allocator before the
        # releaser on the same engine stream -> deadlock.
        if key in prev:
            tile_mod.add_dep_helper(inst.ins, prev[key].ins, sync=True,
                                    reason="rep boundary")
        if "last" in prev and key != "last":
            tile_mod.add_dep_helper(inst.ins, prev["last"].ins, sync=True,
                                    reason="rep boundary")
        return inst

    anchors = {}

    fp32 = mybir.dt.float32
    lg, hid, sh, w1t, out = env["lg"], env["hid"], env["sh"], env["w1t"], env["out"]
    big, sh3, featT, mhT, ident, epsb = (env[k] for k in
                                         ("big", "sh3", "featT", "mhT", "ident", "epsb"))
    idxf, distf, wks, wprime, cw, chidx, rav = (env[k] for k in
        ("idxf", "distf", "wks", "wprime", "cw", "chidx", "rav"))
    b1sb, bwsb, w2sb, cbc = (env[k] for k in ("b1sb", "bwsb", "w2sb", "cbc"))
    zp, Zv, Zi, bwv, rbw, sev, rse, emv, sden, omv, mv, av = (env[k] for k in
        ("zp", "Zv", "Zi", "bwv", "rbw", "sev", "rse", "emv", "sden", "omv",
         "mv", "av"))

    if True:
        if True:
            skipset = env["skipset"]
            # --- phase B: ctx = sum_k searched_hidden (per row-tile) ---
            for t in range(T):
                partials = sbp.tile([P, 2, D], fp32, tag="partials", name=f"partials{t}")
                for h in range(2):
                    shslab = sh3[:, h * (K // 2):(h + 1) * (K // 2), :]
                    if "shdma" not in skipset:
                        nc.sync.dma_start(
                            out=shslab,
                            in_=sh[t * P:(t + 1) * P, h * (K // 2):(h + 1) * (K // 2), :])
                    if "shred" not in skipset:
                        nc.vector.reduce_sum(
                            out=partials[:, h, :], in_=shslab.transpose([0, 2, 1]),
                            axis=mybir.AxisListType.X)
                    else:
                        nc.vector.memset(partials[:, h, :], 0.01)
                nc.vector.tensor_tensor(out=partials[:, 0, :],
                                        in0=partials[:, 0, :],
                                        in1=partials[:, 1, :], op=Alu.add)
                ctx_t = partials[:, 0, :]

                htile = sbp.tile([P, D], fp32, tag="htile", bufs=2, name=f"htile{t}")
                bdep(nc.scalar.dma_start(out=htile[:], in_=hid[t * P:(t + 1) * P, :]),
                     key="fc")
                for c in range(DC):
                    trp = psp.tile([P, P], fp32, tag="trp", name=f"trph{t}_{c}")
                    nc.tensor.transpose(out=trp[:], in_=htile[:, c * P:(c + 1) * P],
                                        identity=ident[:, :])
                    nc.scalar.copy(out=featT[:, c, t * P:(t + 1) * P], in_=trp[:])
                for c in range(DC):
                    trp = psp.tile([P, P], fp32, tag="trp", name=f"trpc{t}_{c}")
                    nc.tensor.transpose(out=trp[:], in_=ctx_t[:, c * P:(c + 1) * P],
                                        identity=ident[:, :])
                    anchors["fc"] = nc.scalar.copy(
                        out=featT[:, DC + c, t * P:(t + 1) * P], in_=trp[:])

            # --- phase C: MLP hidden layer (mhT = relu(w1 @ feat^T + b1)) ---
            for m in range(DC):
                w1sb = sbp.tile([P, FC, P], fp32, tag="w1sb", bufs=1, name=f"w1sb{m}")
                bdep(nc.scalar.dma_start(
                    out=w1sb[:],
                    in_=w1t[:, m * P:(m + 1) * P].rearrange("(c p) j -> p c j", p=P)),
                    key="mm")
                mmp = psp.tile([P, R], fp32, tag="mmp", name=f"mmp{m}")
                kl = range(FC) if "mlp" not in skipset else range(1)
                for k in kl:
                    anchors["mm"] = nc.tensor.matmul(
                        mmp[:], lhsT=w1sb[:, k, :], rhs=featT[:, k, :],
                        start=(k == kl[0]), stop=(k == kl[-1]))
                nc.scalar.activation(out=mhT[:, m, :], in_=mmp[:], func=Act.Relu,
                                     bias=b1sb[:, m:m + 1])

            # --- per-row scalars: bandwidth + mixing ---
            for t in range(T):
                dpb = psp.tile([P, 1], fp32, tag="dotp", name=f"dpb{t}")
                for k in range(FC):
                    nc.tensor.matmul(dpb[:], lhsT=featT[:, k, t * P:(t + 1) * P],
                                     rhs=bwsb[:, k:k + 1],
                                     start=(k == 0), stop=(k == FC - 1))
                nc.scalar.activation(out=bwv[:, t:t + 1], in_=dpb[:], func=Act.Exp,
                                     bias=cbc[:, 0:1])
                dpm = psp.tile([P, 1], fp32, tag="dotp", name=f"dpm{t}")
                for d in range(DC):
                    nc.tensor.matmul(dpm[:], lhsT=mhT[:, d, t * P:(t + 1) * P],
                                     rhs=w2sb[:, d:d + 1],
                                     start=(d == 0), stop=(d == DC - 1))
                nc.scalar.activation(out=emv[:, t:t + 1], in_=dpm[:], func=Act.Exp,
                                     bias=cbc[:, 1:2])

            # mix = em/(1+em); 1-mix = 1/(1+em)
            nc.vector.tensor_scalar_add(out=sden[:], in0=emv[:], scalar1=1.0)
            nc.vector.reciprocal(out=omv[:], in_=sden[:])
            nc.vector.tensor_tensor(out=mv[:], in0=emv[:], in1=omv[:], op=Alu.mult)
            nc.vector.reciprocal(out=rbw[:], in_=bwv[:])

            # knn softmax weights, scaled by mix
            for t in range(T):
                nc.vector.tensor_scalar(
                    out=wks[:, t, :], in0=distf[:, t, :],
                    scalar1=rbw[:, t:t + 1], scalar2=-1.0,
                    op0=Alu.mult, op1=Alu.mult)
                nc.scalar.activation(out=wks[:, t, :], in_=wks[:, t, :], func=Act.Exp,
                                     accum_out=sev[:, t:t + 1])
            nc.vector.reciprocal(out=rse[:], in_=sev[:])
            for t in range(T):
                nc.vector.tensor_scalar(
                    out=wks[:, t, :], in0=wks[:, t, :],
                    scalar1=rse[:, t:t + 1], scalar2=mv[:, t:t + 1],
                    op0=Alu.mult, op1=Alu.mult)

            # duplicate-index combining: wprime[k] = sum_k' [idx_k==idx_k'] wks_k'
            for t in range(T):
                eqm = sbp.tile([P, K, K], fp32, tag="eqm", name=f"eqm{t}")
                bdep(nc.vector.tensor_tensor(
                    out=eqm[:],
                    in0=idxf[:, t, :].unsqueeze(2).to_broadcast([P, K, K]),
                    in1=idxf[:, t, :].unsqueeze(1).to_broadcast([P, K, K]),
                    op=Alu.is_equal), key="wp")
                nc.vector.tensor_tensor(
                    out=eqm[:], in0=eqm[:],
                    in1=wks[:, t, :].unsqueeze(1).to_broadcast([P, K, K]),
                    op=Alu.mult)
                anchors["wp"] = nc.vector.reduce_sum(
                    out=wprime[:, t, :], in_=eqm[:], axis=mybir.AxisListType.X)

            # --- phase D: dense log-softmax-mix over logits ---
            ch, nchv = env["ch"], env["nchv"]
            SCC = env["scc"]     # corr sub-chunk (ping-pong pair)
            nsc = V // SCC       # 8 sub-chunks per tile
            sls = SCC // SC      # 4 local_scatter calls per sub-chunk
            corr = (env["corr_static"] if env["corr_static"] is not None else
                    sbp.tile([P, 2, SCC], fp32, tag="mid", name="corr"))
            assert ch % SCC == 0
            if "ls" in skipset:
                nc.vector.memset(corr[:], 0.0)
            ns2 = ch // SCC
            for t in range(T):
                for c in range(nchv):
                    nc.sync.dma_start(
                        out=big[:, c * ch:(c + 1) * ch],
                        in_=lg[t * P:(t + 1) * P, c * ch:(c + 1) * ch])
                    nc.scalar.activation(
                        out=big[:, c * ch:(c + 1) * ch],
                        in_=big[:, c * ch:(c + 1) * ch], func=Act.Exp,
                        accum_out=zp[:, t * nchv + c:t * nchv + c + 1])
                nc.vector.reduce_sum(out=Zv[:, t:t + 1],
                                     in_=zp[:, t * nchv:(t + 1) * nchv],
                                     axis=mybir.AxisListType.X)
                nc.vector.reciprocal(out=Zi[:, t:t + 1], in_=Zv[:, t:t + 1])
                nc.vector.tensor_tensor(out=av[:, t:t + 1], in0=omv[:, t:t + 1],
                                        in1=Zi[:, t:t + 1], op=Alu.mult)
                nc.vector.tensor_tensor(out=rav[:, t:t + 1], in0=Zv[:, t:t + 1],
                                        in1=sden[:, t:t + 1], op=Alu.mult)
                # software-pipelined: scatters for sub-chunk sc (gpsimd,
                # Z-independent raw wprime values -> can run under the exp
                # pass) ping-pong two corr buffers two sub-chunks ahead of
                # the adds; the add applies the Z*(1+em) factor.
                for sc in range(nsc + 2):
                    a = sc - 2
                    if a >= 0:
                        if "corradd" not in skipset:
                            nc.vector.scalar_tensor_tensor(
                                out=big[:, a * SCC:(a + 1) * SCC],
                                in0=corr[:, a % 2, :],
                                scalar=rav[:, t:t + 1],
                                in1=big[:, a * SCC:(a + 1) * SCC],
                                op0=Alu.mult, op1=Alu.add)
                        if (a + 1) % ns2 == 0:
                            c = a // ns2
                            nc.scalar.activation(
                                out=big[:, c * ch:(c + 1) * ch],
                                in_=big[:, c * ch:(c + 1) * ch], func=Act.Ln,
                                bias=epsb[:], scale=av[:, t:t + 1])
                            anchors["last"] = nc.scalar.dma_start(
                                out=out[t * P:(t + 1) * P, c * ch:(c + 1) * ch],
                                in_=big[:, c * ch:(c + 1) * ch])
                    if sc < nsc and "ls" not in skipset:
                        for i in range(sls):
                            nc.gpsimd.local_scatter(
                                out_ap=corr[:, sc % 2,
                                            i * SC:(i + 1) * SC].bitcast(
                                    mybir.dt.int16),
                                data_ap=wprime[:, t, :].bitcast(mybir.dt.int16),
                                idxs_ap=chidx[:, t, sc * sls + i, :],
                                channels=P, num_elems=2 * SC, num_idxs=2 * K)
    return anchors


def get_nc(reps=1):
    if reps not in _NC:
        _NC[reps] = _build_nc(reps)
    return _NC[reps]


def make_in_maps(hidden, logits, distances, token_indices, searched_hidden,
                 bw_w, bw_b, mw_w1, mw_b1, mw_w2, mw_b2):
    hidden = np.asarray(hidden, dtype=np.float32).reshape(N, D)
    logits = np.asarray(logits, dtype=np.float32).reshape(N, V)
    distances = np.asarray(distances, dtype=np.float32).reshape(N, K)
    tok = np.asarray(token_indices).astype(np.int32).reshape(N, K)
    sh = np.asarray(searched_hidden, dtype=np.float32).reshape(N, K, D)

    # per-1000-chunk relative scatter indices for the int16-halves local_scatter:
    # slot 2k -> 2*(tok - 1000*chunk), slot 2k+1 -> +1; out-of-chunk and
    # duplicate (non-first occurrence) slots get -2.
    eq = tok[:, :, None] == tok[:, None, :]
    isdup = (eq & np.tril(np.ones((K, K), bool), -1)).any(-1)  # (N, K)
    cid = tok // SC
    rel = (tok - cid * SC).astype(np.int16)
    chxf = np.full((N, NSC, 2 * K), -2, np.int16)
    rows_ = np.arange(N)[:, None]
    kk_ = np.arange(K)[None, :]
    chxf[rows_, cid, 2 * kk_] = np.where(isdup, -2, 2 * rel)
    chxf[rows_, cid, 2 * kk_ + 1] = np.where(isdup, -2, 2 * rel + 1)

    w1t = np.ascontiguousarray(np.asarray(mw_w1, np.float32).T)  # [2D, D]
    w1t[D:, :] /= float(K)  # fold the ctx mean's 1/K into the weights
    bwt = np.asarray(bw_w, np.float32).reshape(F, 1).copy()
    bwt[D:, :] /= float(K)
    b1tt = np.ascontiguousarray(np.asarray(mw_b1, np.float32).reshape(DC, P).T)
    w2tt = np.asarray(mw_w2, np.float32).reshape(D, 1).copy()
    cvec = np.array([[float(np.asarray(bw_b).ravel()[0]),
                      float(np.asarray(mw_b2).ravel()[0])]], np.float32)

    in_maps = []
    for c in range(NCORES):
        rs = slice(c * R, (c + 1) * R)
        in_maps.append({
            "lg": np.ascontiguousarray(logits[rs]),
            "hid": np.ascontiguousarray(hidden[rs]),
            "dist": np.ascontiguousarray(distances[rs]),
            "tok": np.ascontiguousarray(tok[rs]),
            "chx": np.ascontiguousarray(
                chxf[rs].reshape(T, P, NSC, 2 * K).transpose(1, 0, 2, 3)),
            "sh": np.ascontiguousarray(sh[rs]),
            "w1t": w1t, "b1t": b1tt, "bwt": bwt, "w2t": w2tt, "cvec": cvec,
        })
    return in_maps


def kernel(**inputs):
    from concourse import bass_utils
    nc = get_nc()
    in_maps = make_in_maps(**inputs)
    res = bass_utils.run_bass_kernel_spmd(nc, in_maps,
                                          core_ids=list(range(NCORES)))
    outp = np.concatenate([res.results[c]["out"] for c in range(NCORES)], axis=0)
    return outp.reshape(B, S, V)



# revision 1
# speedup vs baseline: 1.7587x; 1.7587x over previous
"""Trainium2 Bass kernel for nn_DynamicCombiner (retrieval-kNN combiner).

Computes, per query row n (of N=2048, sharded 256 rows x 8 cores):
    ctx    = mean_k searched_hidden[n]                  [D]
    feat   = [hidden[n], ctx]                           [2D]
    bw     = exp(feat . bw_w + bw_b)
    w      = softmax(-dist[n]/bw)                       [K]
    mhid   = relu(feat @ mw_w1.T + mw_b1)
    mix    = sigmoid(mhid . mw_w2 + mw_b2)
    p      = softmax(logits[n])                         [V]
    out    = log((1-mix)*p + mix*scatter(w at tok[n]) + 1e-10)

Device strategy per core (R=256 rows = 2 partition-tiles of 128):
  - phase B: stream searched_hidden through a shared 128KB/partition SBUF
    slot, DVE-reduce over K -> ctx; TensorE-transpose h and ctx into
    feat^T (the 1/K mean scaling is folded into the host-side weights).
  - phase C: f32 TensorE matmuls for the tiny MLP + the two dot products;
    ACT handles relu/exp; sigmoid is computed as em/(1+em) on DVE to stay
    in the exp/ln ACT table set.
  - phase D: logits row-tile resident in the shared slot; ACT Exp pass with
    accum_out gives Z for free; the vocab-scatter correction is added in
    exp-space (corr = mix*W*Z/(1-mix), so Ln(scale*(t+corr)+eps) lands on
    log((1-mix)p + mix*W + eps)) via gpsimd local_scatter of the f32 values
    as int16 halves into a per-chunk buffer (host pre-builds per-1000-chunk
    relative indices, with out-of-chunk and duplicate slots negative);
    ACT Ln(a*t + eps) with per-partition scale a=(1-mix)/Z then writes the
    final dense values; DMA out. Duplicate token indices get their combined
    weight via a [K,K] is_equal matrix on DVE.
"""

import numpy as np

B, S, D, V, K = 8, 256, 1024, 32000, 32
N = B * S
NCORES = 8
R = N // NCORES  # rows per core
P = 128
T = R // P       # row-tiles per core
F = 2 * D
CH = 8000        # vocab chunk for streaming passes
NCH = V // CH
SC = 1000        # local_scatter sub-chunk (f32 elems; 2000 int16 < 64Ki/32)
NSC = V // SC    # 32 sub-chunks
SPC = CH // SC   # 8 sub-chunks per streaming chunk
DC = D // P      # 8 d-chunks
FC = F // P      # 16 feature chunks
EPS = 1e-10

_NC = {}


def _build_nc(reps=1, skip=()):
    import concourse.bacc as bacc
    import concourse.bass as bass
    import concourse.mybir as mybir
    import concourse.tile as tile
    from concourse.masks import make_identity

    fp32 = mybir.dt.float32
    i32 = mybir.dt.int32
    Alu = mybir.AluOpType
    Act = mybir.ActivationFunctionType

    nc = bacc.Bacc("TRN2", target_bir_lowering=False, debug=False,
                   num_devices=NCORES)

    lg = nc.dram_tensor("lg", [R, V], fp32, kind="ExternalInput")
    hid = nc.dram_tensor("hid", [R, D], fp32, kind="ExternalInput")
    dist = nc.dram_tensor("dist", [R, K], fp32, kind="ExternalInput")
    tok = nc.dram_tensor("tok", [R, K], i32, kind="ExternalInput")
    chx = nc.dram_tensor("chx", [P, T, NSC, 2 * K], mybir.dt.int16,
                         kind="ExternalInput")
    sh = nc.dram_tensor("sh", [R, K, D], fp32, kind="ExternalInput")
    w1t = nc.dram_tensor("w1t", [F, D], fp32, kind="ExternalInput")
    b1t = nc.dram_tensor("b1t", [P, DC], fp32, kind="ExternalInput")
    bwt = nc.dram_tensor("bwt", [F, 1], fp32, kind="ExternalInput")
    w2t = nc.dram_tensor("w2t", [D, 1], fp32, kind="ExternalInput")
    cvec = nc.dram_tensor("cvec", [1, 2], fp32, kind="ExternalInput")
    out = nc.dram_tensor("out", [R, V], fp32, kind="ExternalOutput")

    with tile.TileContext(nc) as tc:
        with (
            tc.tile_pool(name="bigp", bufs=1) as bigp,
            tc.tile_pool(name="sbp", bufs=1) as sbp,
            tc.tile_pool(name="psp", bufs=2, space="PSUM") as psp,
        ):
            # --- static tiles ---
            big = bigp.tile([P, K * D], fp32)          # 128KB/partition slot
            sh3 = big.rearrange("p (k d) -> p k d", k=K)
            featT = sbp.tile([P, FC, R], fp32, tag="mid")
            mhT = sbp.tile([P, DC, R], fp32)
            ident = sbp.tile([P, P], fp32)
            make_identity(nc, ident[:, :])
            epsb = sbp.tile([P, 1], fp32)
            nc.gpsimd.memset(epsb[:], EPS)

            idxi = sbp.tile([P, T, K], i32)
            idxf = sbp.tile([P, T, K], fp32)
            distf = sbp.tile([P, T, K], fp32)
            wks = sbp.tile([P, T, K], fp32)
            wprime = sbp.tile([P, T, K], fp32)
            cw = sbp.tile([P, T, K], fp32)
            chidx = sbp.tile([P, T, NSC, 2 * K], mybir.dt.int16)
            rav = sbp.tile([P, T], fp32)

            b1sb = sbp.tile([P, DC], fp32)
            bwsb = sbp.tile([P, FC], fp32)
            w2sb = sbp.tile([P, DC], fp32)
            cld = sbp.tile([P, 2], fp32)
            cbc = sbp.tile([P, 2], fp32)

            ch = CH if reps == 1 else 4000
            nchv = V // ch
            spc = ch // SC
            scc = 4000 if reps == 1 else 2000
            corr_static = (None if reps == 1 else
                           sbp.tile([P, 2, scc], fp32, name="corrs"))
            zp = sbp.tile([P, T * nchv], fp32)
            Zv = sbp.tile([P, T], fp32)
            Zi = sbp.tile([P, T], fp32)
            bwv = sbp.tile([P, T], fp32)
            rbw = sbp.tile([P, T], fp32)
            sev = sbp.tile([P, T], fp32)
            rse = sbp.tile([P, T], fp32)
            emv = sbp.tile([P, T], fp32)
            sden = sbp.tile([P, T], fp32)
            omv = sbp.tile([P, T], fp32)   # 1 - mix
            mv = sbp.tile([P, T], fp32)    # mix
            av = sbp.tile([P, T], fp32)    # (1-mix)/Z

            # --- small loads (ACT hwdge ring keeps them off the big stream) ---
            nc.scalar.dma_start(out=idxi[:], in_=tok[:, :].rearrange(
                "(t p) k -> p t k", p=P))
            nc.scalar.dma_start(out=distf[:], in_=dist[:, :].rearrange(
                "(t p) k -> p t k", p=P))
            nc.scalar.dma_start(out=chidx[:], in_=chx[:, :, :, :])
            nc.scalar.dma_start(out=b1sb[:], in_=b1t[:, :])
            nc.scalar.dma_start(out=bwsb[:], in_=bwt[:, 0].rearrange(
                "(c p) -> p c", p=P))
            nc.scalar.dma_start(out=w2sb[:], in_=w2t[:, 0].rearrange(
                "(c p) -> p c", p=P))
            nc.scalar.dma_start(out=cld[:1, :], in_=cvec[:, :])
            nc.gpsimd.partition_broadcast(cbc[:], cld[:1, :])

            nc.vector.tensor_copy(idxf[:], idxi[:])

            skipset = set(skip)
            anchors = None
            for rep in range(reps):
                env = dict(locals())
                env["skipset"] = skipset
                if anchors is None:
                    anchors = _emit_body(nc, tc, sbp, psp, mybir, Alu, Act, env)
                else:
                    # Hard barrier between reps: every instruction of this rep
                    # syncs on the previous rep's final store. Prevents the
                    # scheduler from hoisting slot allocators across the rep
                    # boundary (which deadlocks bufs=1 tags).
                    prev_inst = anchors["last"].ins

                    def _barrier_cb(ins_, _prev=prev_inst):
                        tile.add_dep_helper(ins_, _prev, sync=True,
                                            reason="rep barrier")

                    nc._state.push_inst_callback(_barrier_cb)
                    try:
                        anchors = _emit_body(nc, tc, sbp, psp, mybir, Alu,
                                             Act, env)
                    finally:
                        nc._state.remove_inst_callback(_barrier_cb)

    nc.compile()
    return nc


def _emit_body(nc, tc, sbp, psp, mybir, Alu, Act, env):
    import concourse.tile as tile_mod

    prev = env.get("anchors") or {}

    def bdep(inst, key="last"):
        # Cross-rep ordering: tie this rep's first allocator of a bufs-limited
        # tag to the previous rep's instruction whose read releases that tag's
        # slot. Without this the scheduler can order the allocator before the
        # releaser on the same engine stream -> deadlock.
        if key in prev:
            tile_mod.add_dep_helper(inst.ins, prev[key].ins, sync=True,
                                    reason="rep boundary")
        if "last" in prev and key != "last":
            tile_mod.add_dep_helper(inst.ins, prev["last"].ins, sync=True,
                                    reason="rep boundary")
        return inst

    anchors = {}

    fp32 = mybir.dt.float32
    lg, hid, sh, w1t, out = env["lg"], env["hid"], env["sh"], env["w1t"], env["out"]
    big, sh3, featT, mhT, ident, epsb = (env[k] for k in
                                         ("big", "sh3", "featT", "mhT", "ident", "epsb"))
    idxf, distf, wks, wprime, cw, chidx, rav = (env[k] for k in
        ("idxf", "distf", "wks", "wprime", "cw", "chidx", "rav"))
    b1sb, bwsb, w2sb, cbc = (env[k] for k in ("b1sb", "bwsb", "w2sb", "cbc"))
    zp, Zv, Zi, bwv, rbw, sev, rse, emv, sden, omv, mv, av = (env[k] for k in
        ("zp", "Zv", "Zi", "bwv", "rbw", "sev", "rse", "emv", "sden", "omv",
         "mv", "av"))

    if True:
        if True:
            skipset = env["skipset"]
            # --- phase B: ctx = sum_k searched_hidden (per row-tile) ---
            for t in range(T):
                partials = sbp.tile([P, 2, D], fp32, tag="partials", name=f"partials{t}")
                for h in range(2):
                    shslab = sh3[:, h * (K // 2):(h + 1) * (K // 2), :]
                    if "shdma" not in skipset:
                        nc.sync.dma_start(
                            out=shslab,
                            in_=sh[t * P:(t + 1) * P, h * (K // 2):(h + 1) * (K // 2), :])
                    if "shred" not in skipset:
                        nc.vector.reduce_sum(
                            out=partials[:, h, :], in_=shslab.transpose([0, 2, 1]),
                            axis=mybir.AxisListType.X)
                    else:
                        nc.vector.memset(partials[:, h, :], 0.01)
                nc.vector.tensor_tensor(out=partials[:, 0, :],
                                        in0=partials[:, 0, :],
                                        in1=partials[:, 1, :], op=Alu.add)
                ctx_t = partials[:, 0, :]

                htile = sbp.tile([P, D], fp32, tag="htile", bufs=2, name=f"htile{t}")
                bdep(nc.scalar.dma_start(out=htile[:], in_=hid[t * P:(t + 1) * P, :]),
                     key="fc")
                for c in range(DC):
                    trp = psp.tile([P, P], fp32, tag="trp", name=f"trph{t}_{c}")
                    nc.tensor.transpose(out=trp[:], in_=htile[:, c * P:(c + 1) * P],
                                        identity=ident[:, :])
                    nc.scalar.copy(out=featT[:, c, t * P:(t + 1) * P], in_=trp[:])
                for c in range(DC):
                    trp = psp.tile([P, P], fp32, tag="trp", name=f"trpc{t}_{c}")
                    nc.tensor.transpose(out=trp[:], in_=ctx_t[:, c * P:(c + 1) * P],
                                        identity=ident[:, :])
                    anchors["fc"] = nc.scalar.copy(
                        out=featT[:, DC + c, t * P:(t + 1) * P], in_=trp[:])

            # --- phase C: MLP hidden layer (mhT = relu(w1 @ feat^T + b1)) ---
            for m in range(DC):
                w1sb = sbp.tile([P, FC, P], fp32, tag="w1sb", bufs=1, name=f"w1sb{m}")
                bdep(nc.scalar.dma_start(
                    out=w1sb[:],
                    in_=w1t[:, m * P:(m + 1) * P].rearrange("(c p) j -> p c j", p=P)),
                    key="mm")
                mmp = psp.tile([P, R], fp32, tag="mmp", name=f"mmp{m}")
                kl = range(FC) if "mlp" not in skipset else range(1)
                for k in kl:
                    anchors["mm"] = nc.tensor.matmul(
                        mmp[:], lhsT=w1sb[:, k, :], rhs=featT[:, k, :],
                        start=(k == kl[0]), stop=(k == kl[-1]))
                nc.scalar.activation(out=mhT[:, m, :], in_=mmp[:], func=Act.Relu,
                                     bias=b1sb[:, m:m + 1])

            # --- per-row scalars: bandwidth + mixing ---
            for t in range(T):
                dpb = psp.tile([P, 1], fp32, tag="dotp", name=f"dpb{t}")
                for k in range(FC):
                    nc.tensor.matmul(dpb[:], lhsT=featT[:, k, t * P:(t + 1) * P],
                                     rhs=bwsb[:, k:k + 1],
                                     start=(k == 0), stop=(k == FC - 1))
                nc.scalar.activation(out=bwv[:, t:t + 1], in_=dpb[:], func=Act.Exp,
                                     bias=cbc[:, 0:1])
                dpm = psp.tile([P, 1], fp32, tag="dotp", name=f"dpm{t}")
                for d in range(DC):
                    nc.tensor.matmul(dpm[:], lhsT=mhT[:, d, t * P:(t + 1) * P],
                                     rhs=w2sb[:, d:d + 1],
                                     start=(d == 0), stop=(d == DC - 1))
                nc.scalar.activation(out=emv[:, t:t + 1], in_=dpm[:], func=Act.Exp,
                                     bias=cbc[:, 1:2])

            # mix = em/(1+em); 1-mix = 1/(1+em)
            nc.vector.tensor_scalar_add(out=sden[:], in0=emv[:], scalar1=1.0)
            nc.vector.reciprocal(out=omv[:], in_=sden[:])
            nc.vector.tensor_tensor(out=mv[:], in0=emv[:], in1=omv[:], op=Alu.mult)
            nc.vector.reciprocal(out=rbw[:], in_=bwv[:])

            # knn softmax weights, scaled by mix
            for t in range(T):
                nc.vector.tensor_scalar(
                    out=wks[:, t, :], in0=distf[:, t, :],
                    scalar1=rbw[:, t:t + 1], scalar2=-1.0,
                    op0=Alu.mult, op1=Alu.mult)
                nc.scalar.activation(out=wks[:, t, :], in_=wks[:, t, :], func=Act.Exp,
                                     accum_out=sev[:, t:t + 1])
            nc.vector.reciprocal(out=rse[:], in_=sev[:])
            for t in range(T):
                nc.vector.tensor_scalar(
                    out=wks[:, t, :], in0=wks[:, t, :],
                    scalar1=rse[:, t:t + 1], scalar2=mv[:, t:t + 1],
                    op0=Alu.mult, op1=Alu.mult)

            # duplicate-index combining: wprime[k] = sum_k' [idx_k==idx_k'] wks_k'
            for t in range(T):
                eqm = sbp.tile([P, K, K], fp32, tag="eqm", name=f"eqm{t}")
                bdep(nc.vector.tensor_tensor(
                    out=eqm[:],
                    in0=idxf[:, t, :].unsqueeze(2).to_broadcast([P, K, K]),
                    in1=idxf[:, t, :].unsqueeze(1).to_broadcast([P, K, K]),
                    op=Alu.is_equal), key="wp")
                nc.vector.tensor_tensor(
                    out=eqm[:], in0=eqm[:],
                    in1=wks[:, t, :].unsqueeze(1).to_broadcast([P, K, K]),
                    op=Alu.mult)
                anchors["wp"] = nc.vector.reduce_sum(
                    out=wprime[:, t, :], in_=eqm[:], axis=mybir.AxisListType.X)

            # --- phase D: dense log-softmax-mix over logits ---
            ch, nchv = env["ch"], env["nchv"]
            SCC = env["scc"]     # corr sub-chunk (ping-pong pair)
            nsc = V // SCC       # 8 sub-chunks per tile
            sls = SCC // SC      # 4 local_scatter calls per sub-chunk
            corr = (env["corr_static"] if env["corr_static"] is not None else
                    sbp.tile([P, 2, SCC], fp32, tag="mid", name="corr"))
            assert ch % SCC == 0
            if "ls" in skipset:
                nc.vector.memset(corr[:], 0.0)
            ns2 = ch // SCC
            for t in range(T):
                for c in range(nchv):
                    nc.sync.dma_start(
                        out=big[:, c * ch:(c + 1) * ch],
                        in_=lg[t * P:(t + 1) * P, c * ch:(c + 1) * ch])
                    nc.scalar.activation(
                        out=big[:, c * ch:(c + 1) * ch],
                        in_=big[:, c * ch:(c + 1) * ch], func=Act.Exp,
                        accum_out=zp[:, t * nchv + c:t * nchv + c + 1])
                nc.vector.reduce_sum(out=Zv[:, t:t + 1],
                                     in_=zp[:, t * nchv:(t + 1) * nchv],
                                     axis=mybir.AxisListType.X)
                nc.vector.reciprocal(out=Zi[:, t:t + 1], in_=Zv[:, t:t + 1])
                nc.vector.tensor_tensor(out=av[:, t:t + 1], in0=omv[:, t:t + 1],
                                        in1=Zi[:, t:t + 1], op=Alu.mult)
                nc.vector.tensor_tensor(out=rav[:, t:t + 1], in0=Zv[:, t:t + 1],
                                        in1=sden[:, t:t + 1], op=Alu.mult)
                # software-pipelined: scatters for sub-chunk sc (gpsimd,
                # Z-independent raw wprime values -> can run under the exp
                # pass) ping-pong two corr buffers two sub-chunks ahead of
                # the adds; the add applies the Z*(1+em) factor.
                for sc in range(nsc + 2):
                    a = sc - 2
                    if a >= 0:
                        if "corradd" not in skipset:
                            nc.vector.scalar_tensor_tensor(
                                out=big[:, a * SCC:(a + 1) * SCC],
                                in0=corr[:, a % 2, :],
                                scalar=rav[:, t:t + 1],
                                in1=big[:, a * SCC:(a + 1) * SCC],
                                op0=Alu.mult, op1=Alu.add)
                        if (a + 1) % ns2 == 0:
                            c = a // ns2
                            nc.scalar.activation(
                                out=big[:, c * ch:(c + 1) * ch],
                                in_=big[:, c * ch:(c + 1) * ch], func=Act.Ln,
                                bias=epsb[:], scale=av[:, t:t + 1])
                            anchors["last"] = nc.scalar.dma_start(
                                out=out[t * P:(t + 1) * P, c * ch:(c + 1) * ch],
                                in_=big[:, c * ch:(c + 1) * ch])
                    if sc < nsc and "ls" not in skipset:
                        for i in range(sls):
                            nc.gpsimd.local_scatter(
                                out_ap=corr[:, sc % 2,
                                            i * SC:(i + 1) * SC].bitcast(
                                    mybir.dt.int16),
                                data_ap=wprime[:, t, :].bitcast(mybir.dt.int16),
                                idxs_ap=chidx[:, t, sc * sls + i, :],
                                channels=P, num_elems=2 * SC, num_idxs=2 * K)
    return anchors


def get_nc(reps=1):
    if reps not in _NC:
        _NC[reps] = _build_nc(reps)
    return _NC[reps]


def make_in_maps(hidden, logits, distances, token_indices, searched_hidden,
                 bw_w, bw_b, mw_w1, mw_b1, mw_w2, mw_b2):
    hidden = np.asarray(hidden, dtype=np.float32).reshape(N, D)
    logits = np.asarray(logits, dtype=np.float32).reshape(N, V)
    distances = np.asarray(distances, dtype=np.float32).reshape(N, K)
    tok = np.asarray(token_indices).astype(np.int32).reshape(N, K)
    sh = np.asarray(searched_hidden, dtype=np.float32).reshape(N, K, D)

    # per-1000-chunk relative scatter indices for the int16-halves local_scatter:
    # slot 2k -> 2*(tok - 1000*chunk), slot 2k+1 -> +1; out-of-chunk and
    # duplicate (non-first occurrence) slots get -2.
    eq = tok[:, :, None] == tok[:, None, :]
    isdup = (eq & np.tril(np.ones((K, K), bool), -1)).any(-1)  # (N, K)
    cid = tok // SC
    rel = (tok - cid * SC).astype(np.int16)
    chxf = np.full((N, NSC, 2 * K), -2, np.int16)
    rows_ = np.arange(N)[:, None]
    kk_ = np.arange(K)[None, :]
    chxf[rows_, cid, 2 * kk_] = np.where(isdup, -2, 2 * rel)
    chxf[rows_, cid, 2 * kk_ + 1] = np.where(isdup, -2, 2 * rel + 1)

    w1t = np.ascontiguousarray(np.asarray(mw_w1, np.float32).T)  # [2D, D]
    w1t[D:, :] /= float(K)  # fold the ctx mean's 1/K into the weights
    bwt = np.asarray(bw_w, np.float32).reshape(F, 1).copy()
    bwt[D:, :] /= float(K)
    b1tt = np.ascontiguousarray(np.asarray(mw_b1, np.float32).reshape(DC, P).T)
    w2tt = np.asarray(mw_w2, np.float32).reshape(D, 1).copy()
    cvec = np.array([[float(np.asarray(bw_b).ravel()[0]),
                      float(np.asarray(mw_b2).ravel()[0])]], np.float32)

    in_maps = []
    for c in range(NCORES):
        rs = slice(c * R, (c + 1) * R)
        in_maps.append({
            "lg": np.ascontiguousarray(logits[rs]),
            "hid": np.ascontiguousarray(hidden[rs]),
            "dist": np.ascontiguousarray(distances[rs]),
            "tok": np.ascontiguousarray(tok[rs]),
            "chx": np.ascontiguousarray(
                chxf[rs].reshape(T, P, NSC, 2 * K).transpose(1, 0, 2, 3)),
            "sh": np.ascontiguousarray(sh[rs]),
            "w1t": w1t, "b1t": b1tt, "bwt": bwt, "w2t": w2tt, "cvec": cvec,
        })
    return in_maps


def kernel(**inputs):
    from concourse import bass_utils
    nc = get_nc()
    in_maps = make_in_maps(**inputs)
    res = bass_utils.run_bass_kernel_spmd(nc, in_maps,
                                          core_ids=list(range(NCORES)))
    outp = np.concatenate([res.results[c]["out"] for c in range(NCORES)], axis=0)
    return outp.reshape(B, S, V)

